# revision 1
# baseline (speedup 1.0000x reference)
"""Trainium2 Bass kernel for nn_EntityBase (sparse entity attention MLP).

Math (per bs*ts element, 2048 total):
  x1   = relu(x @ W1.T + b1)                       x:[64,128] -> x1:[64,512]
  qkv  = x1 @ Win.T ; q = qkv[:, :512][:16 agents], k, v ; heads 8 x 64
  lg   = (q . k)/8 masked with obs_mask (NEG), softmax over keys,
         fully-masked rows -> 0
  attn = (w @ v) @ Wout.T + b_out, agent-masked to 0
  out  = relu(relu(attn) @ W2.T + b2)              -> [16, 512]

Distribution: data-parallel over the 2048 flattened bs*ts elements across
8 NeuronCores (256 elements/core); weights replicated.

Device dataflow (per core, fully unrolled):
  - activations kept feature-major ([feat, token]); entities transposed via
    PE transpose; all GEMMs fp32r (full-rate PE, ~1e-4 per-product error)
  - attention per pair-of-elements: logitsT [2*64 keys, 8h*2e'*16q] via
    per-head matmuls; softmax over keys done with exp on ACT + ones-column
    matmul partition-reduction + reciprocal + K=1 broadcast matmul;
    cross-element blocks are killed by the host-built NEG bias so the
    pair-packed attnV matmul (K=128 over both elements' keys) is exact
  - attn result accumulated feature-major; Wout GEMM feature-major,
    W2 GEMM emits token-major output directly (no output transpose)
"""
import sys
for _p in ("/opt/trn_rl_repo", "/root/.axon_site/_ro/trn_rl_repo"):
    if _p not in sys.path:
        sys.path.insert(0, _p)

import numpy as np
import concourse.bass as bass
import concourse.tile as tile
from concourse import mybir, bacc
from concourse.bass_utils import run_bass_kernel_spmd

FP32 = mybir.dt.float32
FP32R = mybir.dt.float32r
AF = mybir.ActivationFunctionType
ADD = mybir.AluOpType.add
MULT = mybir.AluOpType.mult

# problem dims (hardcoded per spec)
B, T, NE, ED = 32, 64, 64, 128
NA, E, H, R = 16, 512, 8, 512
HD = E // H
NEG = np.float32(-1e30)
NCORES = 8
BT = B * T                     # 2048
NB = BT // NCORES              # 256 elements per core
NTOK = NB * NE                 # 16384 tokens per core
NAG = NB * NA                  # 4096 agent tokens per core
NSUPER = 8                     # supers per core (32 elements each)
NGROUP = 32                    # groups per core (8 elements each)


def _build_nc():
    nc = bacc.Bacc("TRN2", target_bir_lowering=False, debug=False)
    ap = lambda n, s, d, k: nc.dram_tensor(n, s, d, kind=k).ap()
    ent = ap("ent", [NTOK, ED], FP32R, "ExternalInput")
    w1t = ap("w1t", [ED, E], FP32R, "ExternalInput")        # W1.T
    b1c = ap("b1c", [128, 4], FP32, "ExternalInput")        # b1 chunked
    wqe_d = ap("wqe_d", [E, E], FP32R, "ExternalInput")     # (Win_q/8).T, odd-head cols zeroed
    wqo_d = ap("wqo_d", [E, E], FP32R, "ExternalInput")     # (Win_q/8).T, even-head cols zeroed
    wkt = ap("wkt", [E, E], FP32R, "ExternalInput")         # Win_k.T
    wvt = ap("wvt", [E, E], FP32R, "ExternalInput")         # Win_v.T
    wot = ap("wot", [E, E], FP32R, "ExternalInput")         # Wout.T
    boc = ap("boc", [128, 4], FP32, "ExternalInput")        # b_out chunked
    w2t = ap("w2t", [E, R], FP32R, "ExternalInput")         # W2.T
    b2r = ap("b2r", [1, R], FP32R, "ExternalInput")
    batt = ap("batt", [NB // 2 * 128, 32], FP32, "ExternalInput")   # attn bias per pair
    ntg = ap("ntg", [NSUPER * 128, 512], FP32, "ExternalInput")     # not-agent bcast
    onc = ap("onc", [128, 1], FP32R, "ExternalInput")
    onr = ap("onr", [1, 128], FP32R, "ExternalInput")
    idn = ap("idn", [128, 128], FP32R, "ExternalInput")
    out = ap("out", [NAG, R], FP32, "ExternalOutput")

    with tile.TileContext(nc) as tc:
        with (
            nc.allow_low_precision(reason="fp32r matmul pipeline by design"),
            tc.tile_pool(name="wp", bufs=1) as wp,
            tc.tile_pool(name="act", bufs=2) as act,
            tc.tile_pool(name="xin_p", bufs=4) as xin_p,
            tc.tile_pool(name="small", bufs=4) as small,
            tc.tile_pool(name="ps_big", bufs=3, space="PSUM") as ps_big,
            tc.tile_pool(name="ps_lg", bufs=2, space="PSUM") as ps_lg,
            tc.tile_pool(name="ps_sum", bufs=1, space="PSUM") as ps_sum,
            tc.tile_pool(name="ps_bc", bufs=1, space="PSUM") as ps_bc,
            tc.tile_pool(name="ps_at", bufs=1, space="PSUM") as ps_at,
        ):
            # ---- resident weights/constants ----
            w1s = wp.tile([128, E], FP32R, tag="w1s", name="w1s")
            nc.sync.dma_start(w1s[:], w1t)
            b1s = wp.tile([128, 4], FP32, tag="b1s", name="b1s")
            nc.sync.dma_start(b1s[:], b1c)
            bos = wp.tile([128, 4], FP32, tag="bos", name="bos")
            nc.sync.dma_start(bos[:], boc)
            b2s = wp.tile([1, R], FP32R, tag="b2s", name="b2s")
            nc.sync.dma_start(b2s[:], b2r)
            oc = wp.tile([128, 1], FP32R, tag="oc", name="oc")
            nc.sync.dma_start(oc[:], onc)
            orw = wp.tile([1, 128], FP32R, tag="orw", name="orw")
            nc.sync.dma_start(orw[:], onr)
            ids = wp.tile([128, 128], FP32R, tag="ids", name="ids")
            nc.sync.dma_start(ids[:], idn)
            wqe, wqo, wk, wv, wo, w2 = [], [], [], [], [], []
            for e in range(4):
                for lst, nm, src in ((wqe, "wqe", wqe_d), (wqo, "wqo", wqo_d),
                                     (wk, "wk", wkt),
                                     (wv, "wv", wvt), (wo, "wo", wot),
                                     (w2, "w2", w2t)):
                    t_ = wp.tile([128, 512], FP32R, tag=f"{nm}{e}", name=f"{nm}{e}")
                    nc.sync.dma_start(t_[:], src[e * 128:(e + 1) * 128, :])
                    lst.append(t_)

            for sg in range(NSUPER):
                attnT = [act.tile([128, 512], FP32R, tag=f"attnT{m}", name=f"attnT{m}")
                         for m in range(4)]
                for gg in range(2):                  # 2-group blocks in super
                    x1T = [act.tile([128, 1024], FP32R, tag=f"x1T{m}", name=f"x1T{m}")
                           for m in range(4)]
                    kTs, vts = [], []
                    for sub in range(2):
                        g = sg * 4 + gg * 2 + sub    # global group 0..31
                        # --- load + PE-transpose entities ---
                        xT = act.tile([128, 512], FP32R, tag="xT", name="xT")
                        for c in range(4):
                            xin = xin_p.tile([128, 128], FP32R, tag="xin", name="xin")
                            r0 = g * 512 + c * 128
                            nc.sync.dma_start(xin[:], ent[r0:r0 + 128, :])
                            tp = ps_big.tile([128, 128], FP32R, tag="big", name="big")
                            nc.tensor.transpose(tp[:], xin[:], ids[:])
                            nc.scalar.activation(xT[:, c * 128:(c + 1) * 128],
                                                 tp[:], AF.Copy)
                        # --- fc1: x1T = relu(W1 @ xT + b1) ---
                        for m in range(4):
                            p = ps_big.tile([128, 512], FP32, tag="big", name="big")
                            nc.tensor.matmul(
                                p[:], w1s[:, m * 128:(m + 1) * 128], xT[:])
                            nc.scalar.activation(
                                x1T[m][:, sub * 512:(sub + 1) * 512], p[:],
                                AF.Relu, bias=b1s[:, m:m + 1])
                        # --- kT feature-major ---
                        kT = []
                        for m in range(4):
                            p = ps_big.tile([128, 512], FP32, tag="big", name="big")
                            for e in range(4):
                                nc.tensor.matmul(
                                    p[:], wk[e][:, m * 128:(m + 1) * 128],
                                    x1T[e][:, sub * 512:(sub + 1) * 512],
                                    start=(e == 0), stop=(e == 3))
                            t_ = act.tile([128, 512], FP32R, tag=f"kT{m}", name=f"kT{m}")
                            nc.scalar.activation(t_[:], p[:], AF.Copy)
                            kT.append(t_)
                        kTs.append(kT)
                        # --- v token-major ---
                        vt = []
                        for c in range(4):
                            p = ps_big.tile([128, 512], FP32, tag="big", name="big")
                            for e in range(4):
                                nc.tensor.matmul(
                                    p[:],
                                    x1T[e][:, sub * 512 + c * 128:
                                           sub * 512 + (c + 1) * 128],
                                    wv[e][:], start=(e == 0), stop=(e == 3))
                            t_ = act.tile([128, 512], FP32R, tag=f"v{c}", name=f"v{c}")
                            nc.vector.tensor_copy(t_[:], p[:])
                            vt.append(t_)
                        vts.append(vt)
                    # --- qT for the 2-group (agents only, N=256) ---
                    # two variants with the other head-half zeroed (via the
                    # host-zeroed weight columns) so logits matmuls can use
                    # full K=128 with all operands at base partition 0
                    qTe, qTo = [], []
                    for m in range(4):
                        for wsel, lst, nm in ((wqe, qTe, "qTe"), (wqo, qTo, "qTo")):
                            p = ps_big.tile([128, 256], FP32, tag="big", name="big")
                            for e in range(4):
                                agents = x1T[e][:].rearrange(
                                    "p (el t) -> p el t", el=16)[:, :, 0:NA]
                                nc.tensor.matmul(
                                    p[:], wsel[e][:, m * 128:(m + 1) * 128],
                                    agents, start=(e == 0), stop=(e == 3))
                            t_ = act.tile([128, 256], FP32, tag=f"{nm}{m}",
                                          name=f"{nm}{m}")
                            nc.scalar.activation(t_[:], p[:], AF.Copy)
                            lst.append(t_)
                    # --- attention: 8 pairs in this 2-group block ---
                    for sub in range(2):
                        gl = gg * 2 + sub            # group in super 0..3
                        for pr in range(4):
                            pg = ((sg * 4 + gl) * 4 + pr)    # pair 0..127
                            bia = small.tile([128, 32], FP32, tag="bia", name="bia")
                            nc.sync.dma_start(
                                bia[:], batt[pg * 128:(pg + 1) * 128, :])
                            lg = ps_lg.tile([128, 256], FP32, tag="lg", name="lg")
                            for h in range(8):
                                m = h // 2
                                qv = (qTe if h % 2 == 0 else qTo)[m]
                                nc.tensor.matmul(
                                    lg[:, h * 32:(h + 1) * 32],
                                    kTs[sub][m][:, pr * 128:(pr + 1) * 128
                                                ].bitcast(FP32),
                                    qv[:, sub * 128 + pr * 32:
                                       sub * 128 + (pr + 1) * 32])
                            msk = act.tile([128, 256], FP32, tag="msk", name="msk")
                            nc.vector.tensor_tensor(
                                msk[:].rearrange("p (h q) -> p h q", h=8),
                                lg[:].rearrange("p (h q) -> p h q", h=8),
                                bia[:].unsqueeze(1).broadcast_to([128, 8, 32]),
                                ADD)
                            ex = act.tile([128, 256], FP32R, tag="ex", name="ex")
                            nc.scalar.activation(ex[:], msk[:], AF.Exp)
                            sm = ps_sum.tile([1, 256], FP32, tag="sm", name="sm")
                            nc.tensor.matmul(sm[:], oc[:], ex[:])
                            r1 = small.tile([1, 256], FP32, tag="r1", name="r1")
                            nc.vector.tensor_scalar_add(r1[:], sm[:], 1e-30)
                            r2 = small.tile([1, 256], FP32R, tag="r2", name="r2")
                            nc.vector.reciprocal(r2[:], r1[:])
                            bc = ps_bc.tile([128, 256], FP32, tag="bc", name="bc")
                            nc.tensor.matmul(bc[:], orw[:], r2[:])
                            wn = act.tile([128, 256], FP32, tag="wn", name="wn")
                            nc.vector.tensor_tensor(
                                wn[:], ex[:].bitcast(FP32), bc[:], MULT)
                            # attnV: one MM per head-pair chunk; M=128 packs
                            # both heads' d side-by-side (dst partition 0),
                            # N=64 spans both heads' wn cols; only the
                            # head-diagonal half-blocks are kept.
                            at = ps_at.tile([128, 256], FP32, tag="at", name="at")
                            for m in range(4):
                                nc.tensor.matmul(
                                    at[:, m * 64:(m + 1) * 64],
                                    vts[sub][pr][:, m * 128:(m + 1) * 128
                                                 ].bitcast(FP32),
                                    wn[:, m * 64:(m + 1) * 64])
                            c0 = gl * 128 + pr * 32
                            for m in range(4):
                                nc.vector.tensor_copy(
                                    attnT[m][0:64, c0:c0 + 32],
                                    at[0:64, m * 64:m * 64 + 32])
                                nc.vector.tensor_copy(
                                    attnT[m][64:128, c0:c0 + 32],
                                    at[64:128, m * 64 + 32:m * 64 + 64])
                # --- Wout (feature-major) + post-mask + relu ---
                ntgs = act.tile([128, 512], FP32, tag="ntgs", name="ntgs")
                nc.sync.dma_start(ntgs[:], ntg[sg * 128:(sg + 1) * 128, :])
                sr = []
                for m in range(4):
                    p = ps_big.tile([128, 512], FP32, tag="big", name="big")
                    for e in range(4):
                        nc.tensor.matmul(
                            p[:], wo[e][:, m * 128:(m + 1) * 128],
                            attnT[e][:], start=(e == 0), stop=(e == 3))
                    t_ = act.tile([128, 512], FP32R, tag=f"sr{m}", name=f"sr{m}")
                    nc.scalar.activation(t_[:], p[:], AF.Relu,
                                         bias=bos[:, m:m + 1])
                    nc.vector.tensor_tensor(t_[:], t_[:].bitcast(FP32),
                                            ntgs[:], MULT)
                    sr.append(t_)
                # --- W2 (token-major out) + b2 + relu -> DMA out ---
                for t in range(4):
                    p = ps_big.tile([128, 512], FP32, tag="big", name="big")
                    nc.tensor.matmul(p[:], orw[:], b2s[:],
                                     start=True, stop=False,
                                     skip_group_check=True)
                    for e in range(4):
                        nc.tensor.matmul(
                            p[:], sr[e][:, t * 128:(t + 1) * 128], w2[e][:],
                            start=False, stop=(e == 3), skip_group_check=True)
                    ot = act.tile([128, 512], FP32, tag="ot", name="ot")
                    nc.scalar.activation(ot[:], p[:], AF.Relu)
                    r0 = sg * 512 + t * 128
                    nc.sync.dma_start(out[r0:r0 + 128, :], ot[:])
    nc.compile()
    return nc


_NC_CACHE = None

def _get_nc():
    global _NC_CACHE
    if _NC_CACHE is None:
        _NC_CACHE = _build_nc()
    return _NC_CACHE


def _prep_in_maps(entities, obs_mask, entity_mask, W1, b1, Win, Wout, b_out,
                  W2, b2):
    f32 = np.float32
    ent = np.ascontiguousarray(np.asarray(entities, f32).reshape(BT, NE, ED))
    pre = np.asarray(obs_mask).reshape(BT, NE, NE)[:, :NA, :]   # [2048,16,64]
    agm = np.asarray(entity_mask).reshape(BT, NE)[:, :NA]       # [2048,16]
    W1, b1 = np.asarray(W1, f32), np.asarray(b1, f32)
    Win, Wout = np.asarray(Win, f32), np.asarray(Wout, f32)
    b_out, W2, b2 = np.asarray(b_out, f32), np.asarray(W2, f32), np.asarray(b2, f32)

    wq_t = (Win[0:E] * np.float32(1.0 / np.sqrt(HD))).T   # [e, f]
    fidx = np.arange(E)
    wq_even = wq_t.copy(); wq_even[:, (fidx // HD) % 2 == 1] = 0.0
    wq_odd = wq_t.copy(); wq_odd[:, (fidx // HD) % 2 == 0] = 0.0
    shared = {
        "w1t": np.ascontiguousarray(W1.T),
        "b1c": np.ascontiguousarray(b1.reshape(4, 128).T),
        "wqe_d": np.ascontiguousarray(wq_even),
        "wqo_d": np.ascontiguousarray(wq_odd),
        "wkt": np.ascontiguousarray(Win[E:2 * E].T),
        "wvt": np.ascontiguousarray(Win[2 * E:3 * E].T),
        "wot": np.ascontiguousarray(Wout.T),
        "boc": np.ascontiguousarray(b_out.reshape(4, 128).T),
        "w2t": np.ascontiguousarray(W2.T),
        "b2r": np.ascontiguousarray(b2.reshape(1, R)),
        "onc": np.ones((128, 1), f32),
        "onr": np.ones((1, 128), f32),
        "idn": np.eye(128, dtype=f32),
    }
    in_maps = []
    for c in range(NCORES):
        s = slice(c * NB, (c + 1) * NB)
        ent_c = ent[s].reshape(NTOK, ED)
        # attention bias per pair: [128 pairs, (2e x 64k), (2e' x 16q)]
        obsT = pre[s].astype(f32).transpose(0, 2, 1)      # [256, 64k, 16q]
        bias = np.full((NB // 2, 2, 64, 2, 16), NEG, f32)
        bias[:, 0, :, 0, :] = NEG * obsT[0::2]
        bias[:, 1, :, 1, :] = NEG * obsT[1::2]
        bias = bias.reshape(NB // 2 * 128, 32)
        # not-agent multiplicative mask, replicated over partitions
        ntg_c = (1.0 - agm[s].astype(f32)).reshape(NSUPER, 1, 512)
        ntg_c = np.ascontiguousarray(
            np.broadcast_to(ntg_c, (NSUPER, 128, 512)).reshape(NSUPER * 128, 512))
        m = dict(shared)
        m["ent"] = np.ascontiguousarray(ent_c)
        m["batt"] = np.ascontiguousarray(bias)
        m["ntg"] = ntg_c
        in_maps.append(m)
    return in_maps


def kernel(**inputs) -> np.ndarray:
    nc = _get_nc()
    in_maps = _prep_in_maps(**inputs)
    res = run_bass_kernel_spmd(nc, in_maps, list(range(NCORES)))
    outs = [res.results[c]["out"] for c in range(NCORES)]       # [4096, 512]
    full = np.concatenate(outs, axis=0).reshape(BT, NA, R)
    return np.ascontiguousarray(full.reshape(B, T, NA, R)).astype(np.float32)



# revision 25
# speedup vs baseline: 1041.7866x; 1041.7866x over previous
"""Trainium2 Bass kernel for nn_EntityBase (sparse entity attention MLP).

Math (per bs*ts element, 2048 total):
  x1   = relu(x @ W1.T + b1)                       x:[64,128] -> x1:[64,512]
  qkv  = x1 @ Win.T ; q = qkv[:, :512][:16 agents], k, v ; heads 8 x 64
  lg   = (q . k)/8 masked with obs_mask (NEG), softmax over keys,
         fully-masked rows -> 0
  attn = (w @ v) @ Wout.T + b_out, agent-masked to 0
  out  = relu(relu(attn) @ W2.T + b2)              -> [16, 512]

Distribution: data-parallel over the 2048 flattened bs*ts elements across
8 NeuronCores (256 elements/core); weights replicated.

Device dataflow (per core, 16 blocks of 16 elements):
  - entities transposed on HOST to feature-major [128, 16384]; all big GEMMs
    fp32r with N>=256 (full-rate PE)
  - attention in fp16: kT/vT/qT converted to fp16 during their PSUM->SBUF
    copies; logits per (pair, head) via K=64 PE-tile matmuls (head-slices at
    partition base 0/64), output [128 keys-of-pair, (h,q)] in PSUM
  - softmax per half-group (2 pairs, [128,512]): DVE mask-bias add, ACT exp,
    PE ones-matmul partition sum, DVE eps+reciprocal, PE broadcast matmul,
    DVE normalize to fp16 weights
  - attnV per (pair, head-half): K=128 matmuls writing disjoint PSUM
    partition ranges via tile_position (no extraction copies)
  - Wout feature-major + agent mask (broadcast on device); W2 emits
    token-major output directly
"""
import sys
for _p in ("/opt/trn_rl_repo", "/root/.axon_site/_ro/trn_rl_repo"):
    if _p not in sys.path:
        sys.path.insert(0, _p)

import numpy as np
import concourse.bass as bass
import concourse.tile as tile
from concourse import bass_isa, mybir, bacc
from concourse.bass_utils import run_bass_kernel_spmd

FP32 = mybir.dt.float32
FP32R = mybir.dt.float32r
FP16 = mybir.dt.float16
AF = mybir.ActivationFunctionType
ADD = mybir.AluOpType.add
MULT = mybir.AluOpType.mult

# problem dims (hardcoded per spec)
B, T, NE, ED = 32, 64, 64, 128
NA, E, H, R = 16, 512, 8, 512
HD = E // H
NEG = np.float32(-1e30)
NCORES = 8
BT = B * T                     # 2048
NB = BT // NCORES              # 256 elements per core
NTOK = NB * NE                 # 16384 tokens per core
NAG = NB * NA                  # 4096 agent tokens per core
NBLK = 16                      # blocks per core (16 elements each)


def _build_nc(nrep=1):
    nc = bacc.Bacc("TRN2", target_bir_lowering=False, debug=False)
    ap = lambda n, s, d, k: nc.dram_tensor(n, s, d, kind=k).ap()
    entT = ap("entT", [ED, NTOK], FP32R, "ExternalInput")   # host-transposed
    w1t = ap("w1t", [ED, E], FP32R, "ExternalInput")        # W1.T
    b1c = ap("b1c", [128, 4], FP32, "ExternalInput")        # b1 chunked
    wqt = ap("wqt", [E, E], FP32R, "ExternalInput")         # (Win_q/8).T
    wkt = ap("wkt", [E, E], FP32R, "ExternalInput")         # Win_k.T
    wvt = ap("wvt", [E, E], FP32R, "ExternalInput")         # Win_v.T
    wot = ap("wot", [E, E], FP32R, "ExternalInput")         # Wout.T
    boc = ap("boc", [128, 4], FP32, "ExternalInput")        # b_out chunked
    w2t = ap("w2t", [E, R], FP32R, "ExternalInput")         # W2.T
    b2r = ap("b2r", [1, R], FP32R, "ExternalInput")
    batt = ap("batt", [NBLK * 128, 256], FP32, "ExternalInput")  # mask bias
    ntg = ap("ntg", [1, NBLK * 256], FP32R, "ExternalInput")  # 1-agent_mask
    onc = ap("onc", [128, 1], FP32R, "ExternalInput")
    onr = ap("onr", [1, 128], FP32R, "ExternalInput")
    out = ap("out", [NAG, R], FP32, "ExternalOutput")

    from contextlib import ExitStack
    POOLS = dict(wp=1, entp=3, x1p=2, ktp=2, vtp=3, qp=2, exp_=4, wnp=2,
                 smallp=2, attp=2, srp=1, otp=2, biasp=3, ntgsp=2, khip=2,
                 qhip=2, ntgq=2)
    PSUM_POOLS = dict(ps_big=4, ps_lg=2, ps_at=2)
    with tile.TileContext(nc) as tc:
        with ExitStack() as ctx:
            ctx.enter_context(nc.allow_low_precision(
                reason="fp32r/fp16 matmul pipeline by design"))
            pools = {}
            for nm, bufs in POOLS.items():
                pools[nm] = ctx.enter_context(tc.tile_pool(name=nm, bufs=bufs))
            for nm, bufs in PSUM_POOLS.items():
                pools[nm] = ctx.enter_context(
                    tc.tile_pool(name=nm, bufs=bufs, space="PSUM"))
            aps = dict(entT=entT, w1t=w1t, b1c=b1c, wqt=wqt, wkt=wkt, wvt=wvt,
                       wot=wot, boc=boc, w2t=w2t, b2r=b2r, batt=batt, ntg=ntg,
                       onc=onc, onr=onr, out=out)
            if nrep == 1:
                _emit(nc, pools, aps)
            else:
                with tc.For_i(0, nrep) as _i:
                    _emit(nc, pools, aps)
    nc.compile()
    return nc


def _emit(nc, pools, aps):
    wp, entp, x1p, ktp, vtp, qp = (pools[k] for k in
                                   ("wp", "entp", "x1p", "ktp", "vtp", "qp"))
    exp_, wnp, smallp, attp, srp, otp, biasp = (
        pools[k] for k in ("exp_", "wnp", "smallp", "attp", "srp", "otp",
                           "biasp"))
    ps_big, ps_lg, ps_at, ntgsp = (
        pools[k] for k in ("ps_big", "ps_lg", "ps_at", "ntgsp"))
    khip, qhip, ntgq = pools["khip"], pools["qhip"], pools["ntgq"]
    entT, w1t, b1c, wqt, wkt, wvt, wot, boc, w2t, b2r, batt, ntg, onc, onr, out = (
        aps[k] for k in ("entT", "w1t", "b1c", "wqt", "wkt", "wvt", "wot",
                         "boc", "w2t", "b2r", "batt", "ntg", "onc", "onr",
                         "out"))
    if True:
        if True:
            st = {}   # per-block state for the software pipeline

            def preload(blk):
                ent_t = entp.tile([128, 1024], FP32R, tag="ent", name="ent")
                for hh in range(2):
                    nc.sync.dma_start(
                        ent_t[:, hh * 512:(hh + 1) * 512],
                        entT[:, blk * 1024 + hh * 512:
                             blk * 1024 + (hh + 1) * 512])
                bia = biasp.tile([128, 256], FP32, tag="bia", name="bia")
                nc.sync.dma_start(bia[:], batt[blk * 128:(blk + 1) * 128, :])
                ntr = ntgq.tile([1, 256], FP32R, tag="ntr", name="ntr")
                nc.sync.dma_start(ntr[:], ntg[:, blk * 256:(blk + 1) * 256])
                st[blk] = dict(ent_t=ent_t, bia=bia, ntr=ntr)

            # ---- resident weights/constants, ordered by first use; the
            # first block's inputs are queued before the bulk weights ----
            w1s = wp.tile([128, E], FP32R, tag="w1s", name="w1s")
            nc.sync.dma_start(w1s[:], w1t)
            b1s = wp.tile([128, 4], FP32, tag="b1s", name="b1s")
            nc.sync.dma_start(b1s[:], b1c)
            preload(0)
            wq, wk, wv, wo, w2 = [], [], [], [], []
            wlists = dict(wq=wq, wk=wk, wv=wv, wo=wo, w2=w2)
            for nm, src in (("wk", wkt), ("wv", wvt), ("wq", wqt)):
                for e in range(4):
                    t_ = wp.tile([128, 512], FP32R, tag=f"{nm}{e}",
                                 name=f"{nm}{e}")
                    nc.sync.dma_start(t_[:], src[e * 128:(e + 1) * 128, :])
                    wlists[nm].append(t_)
            oc = wp.tile([128, 1], FP32R, tag="oc", name="oc")
            nc.sync.dma_start(oc[:], onc)
            orw = wp.tile([1, 128], FP32R, tag="orw", name="orw")
            nc.sync.dma_start(orw[:], onr)
            for nm, src in (("wo", wot), ("w2", w2t)):
                for e in range(4):
                    t_ = wp.tile([128, 512], FP32R, tag=f"{nm}{e}",
                                 name=f"{nm}{e}")
                    nc.sync.dma_start(t_[:], src[e * 128:(e + 1) * 128, :])
                    wlists[nm].append(t_)
            bos = wp.tile([128, 4], FP32, tag="bos", name="bos")
            nc.sync.dma_start(bos[:], boc)
            b2s = wp.tile([1, R], FP32R, tag="b2s", name="b2s")
            nc.sync.dma_start(b2s[:], b2r)

            def phase_A(blk):
                # fc1 + k + v + q GEMMs (inputs DMA'd by preload)
                if blk not in st:
                    preload(blk)
                ent_t, bia = st[blk]["ent_t"], st[blk]["bia"]

                x1T = [x1p.tile([128, 1024], FP32R, tag=f"x1T{m}", name=f"x1T{m}")
                       for m in range(4)]
                for h in range(2):
                    for m in range(4):
                        p = ps_big.tile([128, 512], FP32, tag="big", name="big")
                        nc.tensor.matmul(
                            p[:], w1s[:, m * 128:(m + 1) * 128],
                            ent_t[:, h * 512:(h + 1) * 512])
                        nc.scalar.activation(
                            x1T[m][:, h * 512:(h + 1) * 512], p[:],
                            AF.Relu, bias=b1s[:, m:m + 1])

                kT = [[None] * 4 for _ in range(2)]
                vt = [[None] * 4 for _ in range(2)]
                for g in range(2):
                    for m in range(4):
                        p = ps_big.tile([128, 512], FP32, tag="big", name="big")
                        for e in range(4):
                            nc.tensor.matmul(
                                p[:], wk[e][:, m * 128:(m + 1) * 128],
                                x1T[e][:, g * 512:(g + 1) * 512],
                                start=(e == 0), stop=(e == 3))
                        t_ = ktp.tile([128, 512], FP16, tag=f"kT{g}{m}",
                                      name=f"kT{g}{m}")
                        nc.vector.tensor_copy(t_[:], p[:])
                        hi = khip.tile([64, 512], FP16, tag=f"kh{g}{m}",
                                       name=f"kh{g}{m}")
                        nc.sync.dma_start(hi[:], t_[64:128, :])
                        kT[g][m] = (t_, hi)
                    for pr in range(4):
                        p = ps_big.tile([128, 512], FP32, tag="big", name="big")
                        r0 = g * 512 + pr * 128
                        for e in range(4):
                            nc.tensor.matmul(
                                p[:], x1T[e][:, r0:r0 + 128], wv[e][:],
                                start=(e == 0), stop=(e == 3))
                        t_ = vtp.tile([128, 512], FP16, tag=f"vt{g}{pr}",
                                      name=f"vt{g}{pr}")
                        nc.scalar.activation(t_[:], p[:], AF.Copy)
                        vt[g][pr] = t_

                qsb = []
                for m in range(4):
                    pf = ps_big.tile([128, 512], FP32, tag="big", name="big")
                    p = pf[:, 0:256]
                    for e in range(4):
                        agents = x1T[e][:].rearrange(
                            "p (el t) -> p el t", el=16)[:, :, 0:NA]
                        nc.tensor.matmul(
                            p, wq[e][:, m * 128:(m + 1) * 128], agents,
                            start=(e == 0), stop=(e == 3))
                    t_ = qp.tile([128, 256], FP16, tag=f"q{m}", name=f"q{m}")
                    nc.scalar.activation(t_[:], p, AF.Copy)
                    hi = qhip.tile([64, 256], FP16, tag=f"qh{m}", name=f"qh{m}")
                    nc.sync.dma_start(hi[:], t_[64:128, :])
                    qsb.append((t_, hi))
                st[blk].update(kT=kT, vt=vt, qsb=qsb)

            def phase_L(blk):
                # logits + mask bias + exp, 4 half-groups
                import os
                sub = os.environ.get("KSUB", "")
                s = st[blk]
                bia, kT, qsb = s["bia"], s["kT"], s["qsb"]
                lgs, exs = [], []
                for hgi in range(4):
                    g, ph = hgi // 2, hgi % 2
                    lg = ps_lg.tile([128, 512], FP32, tag="lg", name="lg")
                    for pr2 in range(2):
                        p4 = ph * 2 + pr2    # pair in group
                        acol = (g * 8 + p4 * 2) * 16   # agent col base
                        for hh in range(8):
                            m, odd = hh // 2, hh % 2
                            kt_t = kT[g][m][odd]
                            q_t = qsb[m][odd]
                            nc.tensor.matmul(
                                lg[:, pr2 * 256 + hh * 32:
                                   pr2 * 256 + (hh + 1) * 32],
                                kt_t[0:64, p4 * 128:(p4 + 1) * 128],
                                q_t[0:64, acol:acol + 32])
                    if sub == "L1":
                        continue
                    nc.vector.tensor_tensor(
                        lg[:].rearrange("p (pr h q) -> p pr h q", pr=2, h=8),
                        lg[:].rearrange("p (pr h q) -> p pr h q", pr=2, h=8),
                        bia[:, hgi * 64:(hgi + 1) * 64].rearrange(
                            "p (pr q) -> p pr q", pr=2
                        ).unsqueeze(2).broadcast_to([128, 2, 8, 32]),
                        ADD)
                    if sub == "L2":
                        continue
                    ex = exp_.tile([128, 512], FP32R, tag="ex", name="ex")
                    nc.scalar.activation(ex[:], lg[:], AF.Exp)
                    lgs.append(lg)
                    exs.append(ex)
                s["exs"] = exs

            def phase_SCV(blk):
                # softmax sums (gpsimd all-reduce), reciprocals, attnV
                s = st[blk]
                exs, vt = s["exs"], s["vt"]
                wns = []
                for hgi in range(4):
                    dn = smallp.tile([128, 512], FP32, tag="dn", name="dn")
                    nc.gpsimd.partition_all_reduce(
                        dn[:], exs[hgi][:], channels=128,
                        reduce_op=bass_isa.ReduceOp.add)
                    nc.gpsimd.tensor_scalar_add(dn[:], dn[:], 1e-30)
                    r2 = smallp.tile([128, 512], FP32, tag="r2", name="r2")
                    nc.vector.reciprocal(r2[:], dn[:])
                    wn = wnp.tile([128, 512], FP16, tag="wn", name="wn")
                    nc.vector.tensor_tensor(wn[:], exs[hgi][:].bitcast(FP32),
                                            r2[:], MULT)
                    wns.append(wn)
                ats = []
                attnT = [attp.tile([128, 256], FP32R, tag=f"attnT{m}",
                                   name=f"attnT{m}") for m in range(4)]
                for g in range(2):
                    at = ps_at.tile([128, 512], FP32, tag="at", name="at")
                    ats.append(at)
                    for ph in range(2):
                        wn = wns[g * 2 + ph]
                        for pr2 in range(2):
                            p4 = ph * 2 + pr2
                            for m in range(4):
                                nc.tensor.matmul(
                                    at[0:64, m * 128 + p4 * 32:
                                       m * 128 + p4 * 32 + 32],
                                    vt[g][p4][:, m * 128:m * 128 + 64],
                                    wn[:, pr2 * 256 + (2 * m) * 32:
                                       pr2 * 256 + (2 * m) * 32 + 32])
                                nc.tensor.matmul(
                                    at[64:128, m * 128 + p4 * 32:
                                       m * 128 + p4 * 32 + 32],
                                    vt[g][p4][:, m * 128 + 64:(m + 1) * 128],
                                    wn[:, pr2 * 256 + (2 * m + 1) * 32:
                                       pr2 * 256 + (2 * m + 1) * 32 + 32])
                    for m in range(4):
                        nc.scalar.activation(
                            attnT[m][:, g * 128:(g + 1) * 128],
                            at[:, m * 128:(m + 1) * 128], AF.Copy)
                # agent-mask broadcast: [1,256] -> [128,256] -> SBUF
                ntgpf = ps_big.tile([128, 512], FP32, tag="big", name="big")
                nc.tensor.matmul(ntgpf[:, 0:256], orw[:], s["ntr"][:])
                ntg_s = ntgsp.tile([128, 256], FP32, tag="ntg_s", name="ntg_s")
                nc.scalar.activation(ntg_s[:], ntgpf[:, 0:256], AF.Copy)
                s["attnT"] = attnT
                s["ntg_s"] = ntg_s

            def phase_O(blk):
                # Wout + mask, W2 + out DMA
                s = st.pop(blk)
                attnT, ntg_s = s["attnT"], s["ntg_s"]
                sr = []
                for m in range(4):
                    pf = ps_big.tile([128, 512], FP32, tag="big", name="big")
                    p = pf[:, 0:256]
                    for e in range(4):
                        nc.tensor.matmul(
                            p, wo[e][:, m * 128:(m + 1) * 128],
                            attnT[e][:], start=(e == 0), stop=(e == 3))
                    t_ = srp.tile([128, 256], FP32R, tag=f"sr{m}", name=f"sr{m}")
                    nc.scalar.activation(t_[:], p, AF.Relu,
                                         bias=bos[:, m:m + 1])
                    nc.vector.tensor_tensor(t_[:], t_[:].bitcast(FP32),
                                            ntg_s[:], MULT)
                    sr.append(t_)
                for t in range(2):
                    p = ps_big.tile([128, 512], FP32, tag="big", name="big")
                    nc.tensor.matmul(p[:], orw[:], b2s[:],
                                     start=True, stop=False,
                                     skip_group_check=True)
                    for e in range(4):
                        nc.tensor.matmul(
                            p[:], sr[e][:, t * 128:(t + 1) * 128], w2[e][:],
                            start=False, stop=(e == 3), skip_group_check=True)
                    ot = otp.tile([128, 512], FP32, tag="ot", name="ot")
                    nc.scalar.activation(ot[:], p[:], AF.Relu)
                    r0 = blk * 256 + t * 128
                    nc.sync.dma_start(out[r0:r0 + 128, :], ot[:])

            # software pipeline: A(b), SCV(b-1), L(b), O(b-1)
            import os
            nblk = int(os.environ.get("KBLKS", NBLK))
            kphase = os.environ.get("KPHASE", "O")
            phase_A(0)
            if kphase == "A":
                return
            if nblk > 1:
                phase_A(1)
            phase_L(0)
            if kphase == "L":
                return
            if kphase == "SCV":
                phase_SCV(0)
                return
            for blk in range(1, nblk):
                if blk + 1 < nblk:
                    phase_A(blk + 1)
                if blk + 2 < nblk:
                    preload(blk + 2)
                phase_SCV(blk - 1)
                phase_O(blk - 1)
                phase_L(blk)
            phase_SCV(nblk - 1)
            phase_O(nblk - 1)


_NC_CACHE = None

def _get_nc(nrep=1):
    global _NC_CACHE
    if _NC_CACHE is None:
        _NC_CACHE = {}
    if nrep not in _NC_CACHE:
        _NC_CACHE[nrep] = _build_nc(nrep)
    return _NC_CACHE[nrep]


def _prep_in_maps(entities, obs_mask, entity_mask, W1, b1, Win, Wout, b_out,
                  W2, b2):
    f32 = np.float32
    ent = np.asarray(entities, f32).reshape(BT, NE, ED)
    pre = np.asarray(obs_mask).reshape(BT, NE, NE)[:, :NA, :]   # [2048,16,64]
    agm = np.asarray(entity_mask).reshape(BT, NE)[:, :NA]       # [2048,16]
    W1, b1 = np.asarray(W1, f32), np.asarray(b1, f32)
    Win, Wout = np.asarray(Win, f32), np.asarray(Wout, f32)
    b_out, W2, b2 = np.asarray(b_out, f32), np.asarray(W2, f32), np.asarray(b2, f32)

    shared = {
        "w1t": np.ascontiguousarray(W1.T),
        "b1c": np.ascontiguousarray(b1.reshape(4, 128).T),
        "wqt": np.ascontiguousarray((Win[0:E] * np.float32(1.0 / np.sqrt(HD))).T),
        "wkt": np.ascontiguousarray(Win[E:2 * E].T),
        "wvt": np.ascontiguousarray(Win[2 * E:3 * E].T),
        "wot": np.ascontiguousarray(Wout.T),
        "boc": np.ascontiguousarray(b_out.reshape(4, 128).T),
        "w2t": np.ascontiguousarray(W2.T),
        "b2r": np.ascontiguousarray(b2.reshape(1, R)),
        "onc": np.ones((128, 1), f32),
        "onr": np.ones((1, 128), f32),
    }
    in_maps = []
    for c in range(NCORES):
        s = slice(c * NB, (c + 1) * NB)
        entT_c = np.ascontiguousarray(
            ent[s].reshape(NTOK, ED).T)                   # [128, 16384]
        # attention bias per pair: [128 (2e x 64k), (2e' x 16q)]
        obsT = pre[s].astype(f32).transpose(0, 2, 1)      # [256, 64k, 16q]
        bias = np.full((NB // 2, 2, 64, 2, 16), NEG, f32)
        bias[:, 0, :, 0, :] = NEG * obsT[0::2]
        bias[:, 1, :, 1, :] = NEG * obsT[1::2]
        bias = bias.reshape(NB // 2, 128, 32)             # [128 pairs,128,32]
        # regroup to per-block [16, 128, (8 pairs x 32)]
        bias = np.ascontiguousarray(
            bias.reshape(NBLK, 8, 128, 32).transpose(0, 2, 1, 3)
        ).reshape(NBLK * 128, 256)
        ntg_c = np.ascontiguousarray(
            (1.0 - agm[s].astype(f32)).reshape(1, NBLK * 256))
        m = dict(shared)
        m["entT"] = entT_c
        m["batt"] = bias
        m["ntg"] = ntg_c
        in_maps.append(m)
    return in_maps


def kernel(**inputs) -> np.ndarray:
    nc = _get_nc()
    in_maps = _prep_in_maps(**inputs)
    res = run_bass_kernel_spmd(nc, in_maps, list(range(NCORES)))
    outs = [res.results[c]["out"] for c in range(NCORES)]       # [4096, 512]
    full = np.concatenate(outs, axis=0).reshape(BT, NA, R)
    return np.ascontiguousarray(full.reshape(B, T, NA, R)).astype(np.float32)
